# revision 2
# baseline (speedup 1.0000x reference)
"""LocalFeatureAggregation Trainium2 kernel (8 NeuronCores, data-parallel).

v2 -> v4: gathers as 2x1024-idx calls on 2 SWDGE queues (halved Pool dispatch),
hoisted one sup ahead; all PSUM evacuations (relu/exp) on Act; pt = E*x and
t = psr*rec on Pool; k=16 segmented sums as in-place bf16 tree-adds on DVE
(L1 per half for latency, f32 final level); parity select on DVE.
"""

import numpy as np
import ml_dtypes

import concourse.bass as bass
import concourse.bacc as bacc
import concourse.tile as tile
from concourse import mybir
from concourse.bass_utils import run_bass_kernel_spmd

BN_EPS = 1e-5
P = 128
N_NODES = 50000
K_NBR = 16
C_IN = 128
C2 = 256
C_OUT = 128
N_CORES = 8
NODES_PC_RAW = N_NODES // N_CORES
SUP = 2048
GCALL = 512
NODES_PER_SUP = SUP // K_NBR                # 128
HALF = 1024
BF16 = mybir.dt.bfloat16
F32 = mybir.dt.float32

_PROG_CACHE = {}


def build_program(nsup: int):
    nodes_pc = nsup * NODES_PER_SUP
    nc = bacc.Bacc("TRN2", num_devices=N_CORES)

    tab_d = nc.dram_tensor("tab", [N_NODES // 2, 2 * C_IN], BF16, kind="ExternalInput")
    idx_d = nc.dram_tensor("idxs", [nsup * P, SUP // 16], mybir.dt.int16,
                           kind="ExternalInput")
    pm_d = nc.dram_tensor("pmask", [nsup, SUP], mybir.dt.uint8, kind="ExternalInput")
    w1_d = nc.dram_tensor("w1", [C_IN, C2], BF16, kind="ExternalInput")
    ws_d = nc.dram_tensor("wsp", [C2, C2], BF16, kind="ExternalInput")
    wm_d = nc.dram_tensor("wmp", [C2, C_OUT], BF16, kind="ExternalInput")
    b1_d = nc.dram_tensor("b1c", [P, 2], F32, kind="ExternalInput")
    cs_d = nc.dram_tensor("csc", [P, 2], F32, kind="ExternalInput")
    bm_d = nc.dram_tensor("bmr", [1, C_OUT], BF16, kind="ExternalInput")
    out_d = nc.dram_tensor("out", [nodes_pc, C_OUT], F32, kind="ExternalOutput")

    with tile.TileContext(nc) as tc:
        with (
            tc.tile_pool(name="consts", bufs=1) as consts,
            tc.tile_pool(name="gsup", bufs=6) as gsup,
            tc.tile_pool(name="sel", bufs=16) as selp,
            tc.tile_pool(name="work", bufs=3) as work,
            tc.tile_pool(name="small", bufs=4) as small,
            tc.tile_pool(name="outp", bufs=3) as outp,
            tc.tile_pool(name="psum", bufs=3, space="PSUM") as psum,
            tc.tile_pool(name="psout", bufs=2, space="PSUM") as psout,
        ):
            # ---- constants -------------------------------------------------
            w1s = consts.tile([P, C2], BF16)
            nc.sync.dma_start(out=w1s, in_=w1_d[:, :])
            wss = consts.tile([P, 2, C2], BF16)
            for k in range(2):
                nc.sync.dma_start(out=wss[:, k, :], in_=ws_d[k * P:(k + 1) * P, :])
            wms = consts.tile([P, 2, C_OUT], BF16)
            for k in range(2):
                nc.sync.dma_start(out=wms[:, k, :], in_=wm_d[k * P:(k + 1) * P, :])
            b1c = consts.tile([P, 2], F32)
            nc.sync.dma_start(out=b1c, in_=b1_d[:, :])
            csc = consts.tile([P, 2], F32)
            nc.sync.dma_start(out=csc, in_=cs_d[:, :])
            bmb = consts.tile([1, C_OUT], BF16)
            nc.sync.dma_start(out=bmb, in_=bm_d[:, :])
            ones1 = consts.tile([1, P], BF16)
            nc.vector.memset(ones1, 1.0)

            def do_gathers(s):
                idx_t = gsup.tile([P, SUP // 16], mybir.dt.int16)
                nc.sync.dma_start(out=idx_t, in_=idx_d[s * P:(s + 1) * P, :])
                pm_t = gsup.tile([P, SUP], mybir.dt.uint8)
                nc.sync.dma_start(
                    out=pm_t,
                    in_=bass.AP(tensor=pm_d, offset=s * SUP, ap=[[0, P], [1, SUP]]),
                )
                fgq = []
                for q in range(SUP // GCALL):
                    fg = selp.tile([P, 2, GCALL], BF16, tag="fg")
                    nc.gpsimd.dma_gather(
                        fg[:, :, :], tab_d[:, :],
                        idx_t[:, q * (GCALL // 16):(q + 1) * (GCALL // 16)],
                        num_idxs=GCALL, num_idxs_reg=GCALL, elem_size=2 * C_IN,
                        transpose=True,
                    )
                    nc.vector.copy_predicated(
                        fg[:, 0, :], pm_t[:, q * GCALL:(q + 1) * GCALL],
                        fg[:, 1, :],
                    )
                    fgq.append(fg)
                return fgq

            fg_pipe = [do_gathers(0), do_gathers(1)]
            for s in range(nsup):
                fgq = fg_pipe.pop(0)
                if s + 2 < nsup:
                    fg_pipe.append(do_gathers(s + 2))

                # combined tile: [p, (et_m0|et_m1|pt_m0|pt_m1), SUP]
                ept = work.tile([P, 4, SUP], BF16, tag="ept")
                xgt = work.tile([P, 2, SUP], BF16, tag="xgt")
                eptv = ept.rearrange("p w (n k) -> p w n k", k=K_NBR)

                for h in range(SUP // HALF):
                    cols = slice(h * HALF, (h + 1) * HALF)
                    ncol = slice(h * (HALF // K_NBR), (h + 1) * (HALF // K_NBR))
                    # x = relu(W1.T @ f + b1), channel-major, bf16
                    for m in range(2):
                        xg_ps = psum.tile([P, HALF], F32, tag="mm")
                        for q in range(0, HALF, 512):
                            c0 = h * HALF + q
                            nc.tensor.matmul(
                                xg_ps[:, q:q + 512],
                                lhsT=w1s[:, m * P:(m + 1) * P],
                                rhs=fgq[c0 // GCALL][:, 0, c0 % GCALL:c0 % GCALL + 512],
                                start=True, stop=True,
                            )
                        if m == 1 and h == 1:
                            nc.scalar.activation(
                                out=xgt[:, m, h * HALF:h * HALF + 512],
                                in_=xg_ps[:, 0:512],
                                func=mybir.ActivationFunctionType.Relu,
                                bias=b1c[:, m:m + 1], scale=1.0,
                            )
                            nc.vector.tensor_scalar(
                                out=xgt[:, m, h * HALF + 512:(h + 1) * HALF],
                                in0=xg_ps[:, 512:HALF],
                                scalar1=b1c[:, m:m + 1], scalar2=0.0,
                                op0=mybir.AluOpType.add, op1=mybir.AluOpType.max,
                            )
                        else:
                            nc.scalar.activation(
                                out=xgt[:, m, cols], in_=xg_ps,
                                func=mybir.ActivationFunctionType.Relu,
                                bias=b1c[:, m:m + 1], scale=1.0,
                            )
                    # logits -> E = exp(x @ Ws' + cs), bf16
                    for m in range(2):
                        l_ps = psum.tile([P, HALF], F32, tag="mm")
                        for q in range(0, HALF, 512):
                            for k in range(2):
                                nc.tensor.matmul(
                                    l_ps[:, q:q + 512],
                                    lhsT=wss[:, k, m * P:(m + 1) * P],
                                    rhs=xgt[:, k, h * HALF + q:h * HALF + q + 512],
                                    start=(k == 0), stop=(k == 1),
                                )
                        nc.scalar.activation(
                            out=ept[:, m, cols], in_=l_ps,
                            func=mybir.ActivationFunctionType.Exp,
                            bias=csc[:, m:m + 1], scale=1.0,
                        )
                    # pt = E * x  (Pool)
                    nc.gpsimd.tensor_tensor(
                        out=ept[:, 2:4, cols], in0=ept[:, 0:2, cols],
                        in1=xgt[:, :, cols], op=mybir.AluOpType.mult,
                    )
                    # tree L1 for this half (DVE): k 16 -> 8 in place
                    nc.vector.tensor_tensor(
                        out=eptv[:, :, ncol, 0:8], in0=eptv[:, :, ncol, 0:8],
                        in1=eptv[:, :, ncol, 8:16], op=mybir.AluOpType.add,
                    )

                # ---- remaining tree levels (DVE), f32 last -----------------
                nc.vector.tensor_tensor(
                    out=eptv[:, :, :, 0:4], in0=eptv[:, :, :, 0:4],
                    in1=eptv[:, :, :, 4:8], op=mybir.AluOpType.add,
                )
                nc.vector.tensor_tensor(
                    out=eptv[:, :, :, 0:2], in0=eptv[:, :, :, 0:2],
                    in1=eptv[:, :, :, 2:4], op=mybir.AluOpType.add,
                )
                epr = small.tile([P, 4, NODES_PER_SUP], F32, tag="epr")
                nc.gpsimd.tensor_tensor(
                    out=epr, in0=eptv[:, :, :, 0], in1=eptv[:, :, :, 1],
                    op=mybir.AluOpType.add,
                )
                rec = small.tile([P, 2, NODES_PER_SUP], F32, tag="rec")
                nc.vector.reciprocal_approx_fast(
                    rec.rearrange("p m n -> p (m n)"),
                    epr[:, 0:2, :].rearrange("p m n -> p (m n)"),
                )
                tacc = outp.tile([P, 2, NODES_PER_SUP], BF16)
                nc.gpsimd.tensor_tensor(
                    out=tacc, in0=epr[:, 2:4, :], in1=rec,
                    op=mybir.AluOpType.mult,
                )

                # out[nodes, :] = t.T @ Wm' + bm'
                o_ps = psout.tile([P, C_OUT], F32, tag="o")
                for k in range(2):
                    nc.tensor.matmul(
                        o_ps, lhsT=tacc[:, k, :], rhs=wms[:, k, :],
                        start=(k == 0), stop=False,
                    )
                nc.tensor.matmul(
                    o_ps, lhsT=ones1, rhs=bmb, start=False, stop=True,
                )
                oo = outp.tile([P, C_OUT], F32)
                nc.vector.tensor_copy(out=oo, in_=o_ps)
                nc.sync.dma_start(
                    out=out_d[s * NODES_PER_SUP:(s + 1) * NODES_PER_SUP, :], in_=oo
                )

    nc.compile()
    return nc


def _get_prog(nsup: int):
    if nsup not in _PROG_CACHE:
        _PROG_CACHE[nsup] = build_program(nsup)
    return _PROG_CACHE[nsup]


def prep_inputs(features, neighbor_idx, W1, b1, gamma, beta, run_mean, run_var,
                Ws, Wm, bm, n_cores=N_CORES):
    bf16 = ml_dtypes.bfloat16
    a = (gamma / np.sqrt(run_var + BN_EPS)).astype(np.float32)
    c = (beta - run_mean * a).astype(np.float32)
    wsp = (a[:, None] * Ws).astype(bf16)
    csv = (c @ Ws).astype(np.float32)
    wmp = (a[:, None] * Wm).astype(bf16)
    bmv = (c @ Wm + bm).astype(np.float32)

    tab = np.ascontiguousarray(features.astype(bf16).reshape(N_NODES // 2, 2 * C_IN))
    w1b = np.ascontiguousarray(W1.astype(bf16))
    b1c = np.ascontiguousarray(b1.astype(np.float32).reshape(2, P).T)
    csc = np.ascontiguousarray(csv.reshape(2, P).T)
    bmr = bmv.astype(bf16).reshape(1, C_OUT)

    n_pc = neighbor_idx.shape[0] // n_cores
    nodes_pc = -(-n_pc // NODES_PER_SUP) * NODES_PER_SUP
    nsup = nodes_pc // NODES_PER_SUP

    shared = dict(tab=tab, w1=w1b, wsp=np.ascontiguousarray(wsp),
                  wmp=np.ascontiguousarray(wmp), b1c=b1c, csc=csc, bmr=bmr)
    in_maps = []
    for ci in range(n_cores):
        ni = neighbor_idx[ci * n_pc:(ci + 1) * n_pc].astype(np.int64)
        if nodes_pc != n_pc:
            ni = np.concatenate(
                [ni, np.zeros((nodes_pc - n_pc, K_NBR), dtype=np.int64)], axis=0)
        flat = ni.reshape(-1)
        pair = (flat >> 1).astype(np.int16)
        parity = (flat & 1).astype(np.uint8)
        wrapped = pair.reshape(nsup, SUP // 16, 16)
        wrapped = np.transpose(wrapped, (0, 2, 1))
        idxs = np.tile(wrapped, (1, 8, 1)).reshape(nsup * P, SUP // 16)
        in_maps.append(dict(shared,
                            idxs=np.ascontiguousarray(idxs),
                            pmask=np.ascontiguousarray(parity.reshape(nsup, SUP))))
    return in_maps, nsup, n_pc


def kernel(**inputs):
    in_maps, nsup, n_pc = prep_inputs(**inputs)
    nc = _get_prog(nsup)
    res = run_bass_kernel_spmd(nc, in_maps, core_ids=list(range(N_CORES)))
    return np.concatenate([r["out"][:n_pc] for r in res.results], axis=0)


# revision 3
# speedup vs baseline: 1.0971x; 1.0971x over previous
"""LocalFeatureAggregation Trainium2 kernel (8 NeuronCores, data-parallel over nodes).

Algorithm (reference):
    x = relu(features @ W1 + b1)            # (N, 2C)
    g = BN(x[neighbor_idx])                 # (N, k, 2C), inference BN
    s = softmax(g @ Ws, axis=k)             # (N, k, 2C)
    out = (sum_k s * g) @ Wm + bm           # (N, C_out)

Device strategy (v2, ~2x faster than the tensor_reduce baseline):
  - BN folds into weights host-side; softmax needs no max-subtraction.
  - Data-parallel: core i handles 6250 nodes (padded to 6272).  Feature
    table replicated as bf16 DRAM pairs (int16 gather idx = node>>1), the
    odd/even 128-ch half selected on-chip with copy_predicated (DVE).
  - Channel-major throughout; per-sup pipeline (2048 gathered cols):
      gather (Pool dispatch, hoisted 2 sups ahead)
      -> parity select (DVE) -> W1 matmul (PE)
      -> bias+relu PSUM evac (m0: Act, m1: DVE tensor_scalar)
      -> Ws matmul (PE) -> exp+bias evac (Act)
      -> pt = E*x (Pool) -> k=16 segmented sums as in-place bf16
         tree-adds (DVE, L1 per half; last level f32 on Pool)
      -> reciprocal (DVE), t = psr*rec (Pool)
      -> out matmul with bias via 1-partition ones-row pass (PE)
      -> PSUM evac (Act copy) -> DMA out.
  - Engine assignment tuned against the CoreSim cost model: DVE ~95%
    busy is the bound; GPSIMD cannot touch PSUM on HW (verifier).
"""

import numpy as np
import ml_dtypes

import concourse.bass as bass
import concourse.bacc as bacc
import concourse.tile as tile
from concourse import mybir
from concourse.bass_utils import run_bass_kernel_spmd

BN_EPS = 1e-5
P = 128
N_NODES = 50000
K_NBR = 16
C_IN = 128
C2 = 256
C_OUT = 128
N_CORES = 8
NODES_PC_RAW = N_NODES // N_CORES
SUP = 2048
GCALL = 512
NODES_PER_SUP = SUP // K_NBR                # 128
HALF = 1024
BF16 = mybir.dt.bfloat16
F32 = mybir.dt.float32

_PROG_CACHE = {}


def build_program(nsup: int):
    nodes_pc = nsup * NODES_PER_SUP
    nc = bacc.Bacc("TRN2", num_devices=N_CORES)

    tab_d = nc.dram_tensor("tab", [N_NODES // 2, 2 * C_IN], BF16, kind="ExternalInput")
    idx_d = nc.dram_tensor("idxs", [nsup * P, SUP // 16], mybir.dt.int16,
                           kind="ExternalInput")
    pm_d = nc.dram_tensor("pmask", [nsup, SUP], mybir.dt.uint8, kind="ExternalInput")
    w1_d = nc.dram_tensor("w1", [C_IN, C2], BF16, kind="ExternalInput")
    ws_d = nc.dram_tensor("wsp", [C2, C2], BF16, kind="ExternalInput")
    wm_d = nc.dram_tensor("wmp", [C2, C_OUT], BF16, kind="ExternalInput")
    b1_d = nc.dram_tensor("b1c", [P, 2], F32, kind="ExternalInput")
    cs_d = nc.dram_tensor("csc", [P, 2], F32, kind="ExternalInput")
    bm_d = nc.dram_tensor("bmr", [1, C_OUT], BF16, kind="ExternalInput")
    out_d = nc.dram_tensor("out", [nodes_pc, C_OUT], F32, kind="ExternalOutput")

    with tile.TileContext(nc) as tc:
        with (
            tc.tile_pool(name="consts", bufs=1) as consts,
            tc.tile_pool(name="gsup", bufs=6) as gsup,
            tc.tile_pool(name="sel", bufs=16) as selp,
            tc.tile_pool(name="work", bufs=3) as work,
            tc.tile_pool(name="small", bufs=4) as small,
            tc.tile_pool(name="outp", bufs=3) as outp,
            tc.tile_pool(name="psum", bufs=3, space="PSUM") as psum,
            tc.tile_pool(name="psout", bufs=2, space="PSUM") as psout,
        ):
            # ---- constants -------------------------------------------------
            w1s = consts.tile([P, C2], BF16)
            nc.sync.dma_start(out=w1s, in_=w1_d[:, :])
            wss = consts.tile([P, 2, C2], BF16)
            for k in range(2):
                nc.sync.dma_start(out=wss[:, k, :], in_=ws_d[k * P:(k + 1) * P, :])
            wms = consts.tile([P, 2, C_OUT], BF16)
            for k in range(2):
                nc.sync.dma_start(out=wms[:, k, :], in_=wm_d[k * P:(k + 1) * P, :])
            b1c = consts.tile([P, 2], F32)
            nc.sync.dma_start(out=b1c, in_=b1_d[:, :])
            csc = consts.tile([P, 2], F32)
            nc.sync.dma_start(out=csc, in_=cs_d[:, :])
            bmb = consts.tile([1, C_OUT], BF16)
            nc.sync.dma_start(out=bmb, in_=bm_d[:, :])
            ones1 = consts.tile([1, P], BF16)
            nc.vector.memset(ones1, 1.0)

            def do_gathers(s):
                idx_t = gsup.tile([P, SUP // 16], mybir.dt.int16)
                nc.sync.dma_start(out=idx_t, in_=idx_d[s * P:(s + 1) * P, :])
                pm_t = gsup.tile([P, SUP], mybir.dt.uint8)
                nc.sync.dma_start(
                    out=pm_t,
                    in_=bass.AP(tensor=pm_d, offset=s * SUP, ap=[[0, P], [1, SUP]]),
                )
                fgq = []
                for q in range(SUP // GCALL):
                    fg = selp.tile([P, 2, GCALL], BF16, tag="fg")
                    nc.gpsimd.dma_gather(
                        fg[:, :, :], tab_d[:, :],
                        idx_t[:, q * (GCALL // 16):(q + 1) * (GCALL // 16)],
                        num_idxs=GCALL, num_idxs_reg=GCALL, elem_size=2 * C_IN,
                        transpose=True,
                    )
                    nc.vector.copy_predicated(
                        fg[:, 0, :], pm_t[:, q * GCALL:(q + 1) * GCALL],
                        fg[:, 1, :],
                    )
                    fgq.append(fg)
                return fgq

            fg_pipe = [do_gathers(0), do_gathers(1)]
            for s in range(nsup):
                fgq = fg_pipe.pop(0)
                if s + 2 < nsup:
                    fg_pipe.append(do_gathers(s + 2))

                # combined tile: [p, (et_m0|et_m1|pt_m0|pt_m1), SUP]
                ept = work.tile([P, 4, SUP], BF16, tag="ept")
                xgt = work.tile([P, 2, SUP], BF16, tag="xgt")
                eptv = ept.rearrange("p w (n k) -> p w n k", k=K_NBR)

                for h in range(SUP // HALF):
                    cols = slice(h * HALF, (h + 1) * HALF)
                    ncol = slice(h * (HALF // K_NBR), (h + 1) * (HALF // K_NBR))
                    # x = relu(W1.T @ f + b1), channel-major, bf16
                    for m in range(2):
                        xg_ps = psum.tile([P, HALF], F32, tag="mm")
                        for q in range(0, HALF, 512):
                            c0 = h * HALF + q
                            nc.tensor.matmul(
                                xg_ps[:, q:q + 512],
                                lhsT=w1s[:, m * P:(m + 1) * P],
                                rhs=fgq[c0 // GCALL][:, 0, c0 % GCALL:c0 % GCALL + 512],
                                start=True, stop=True,
                            )
                        if m == 1:
                            nc.vector.tensor_scalar(
                                out=xgt[:, m, cols], in0=xg_ps,
                                scalar1=b1c[:, m:m + 1], scalar2=0.0,
                                op0=mybir.AluOpType.add, op1=mybir.AluOpType.max,
                            )
                        else:
                            nc.scalar.activation(
                                out=xgt[:, m, cols], in_=xg_ps,
                                func=mybir.ActivationFunctionType.Relu,
                                bias=b1c[:, m:m + 1], scale=1.0,
                            )
                    # logits -> E = exp(x @ Ws' + cs), bf16
                    for m in range(2):
                        l_ps = psum.tile([P, HALF], F32, tag="mm")
                        for q in range(0, HALF, 512):
                            for k in range(2):
                                nc.tensor.matmul(
                                    l_ps[:, q:q + 512],
                                    lhsT=wss[:, k, m * P:(m + 1) * P],
                                    rhs=xgt[:, k, h * HALF + q:h * HALF + q + 512],
                                    start=(k == 0), stop=(k == 1),
                                )
                        nc.scalar.activation(
                            out=ept[:, m, cols], in_=l_ps,
                            func=mybir.ActivationFunctionType.Exp,
                            bias=csc[:, m:m + 1], scale=1.0,
                        )
                    # pt = E * x  (Pool)
                    nc.gpsimd.tensor_tensor(
                        out=ept[:, 2:4, cols], in0=ept[:, 0:2, cols],
                        in1=xgt[:, :, cols], op=mybir.AluOpType.mult,
                    )
                    # tree L1 for this half (DVE): k 16 -> 8 in place
                    nc.vector.tensor_tensor(
                        out=eptv[:, :, ncol, 0:8], in0=eptv[:, :, ncol, 0:8],
                        in1=eptv[:, :, ncol, 8:16], op=mybir.AluOpType.add,
                    )

                # ---- remaining tree levels (DVE), f32 last -----------------
                nc.vector.tensor_tensor(
                    out=eptv[:, :, :, 0:4], in0=eptv[:, :, :, 0:4],
                    in1=eptv[:, :, :, 4:8], op=mybir.AluOpType.add,
                )
                nc.vector.tensor_tensor(
                    out=eptv[:, :, :, 0:2], in0=eptv[:, :, :, 0:2],
                    in1=eptv[:, :, :, 2:4], op=mybir.AluOpType.add,
                )
                epr = small.tile([P, 4, NODES_PER_SUP], F32, tag="epr")
                nc.gpsimd.tensor_tensor(
                    out=epr, in0=eptv[:, :, :, 0], in1=eptv[:, :, :, 1],
                    op=mybir.AluOpType.add,
                )
                rec = small.tile([P, 2, NODES_PER_SUP], F32, tag="rec")
                nc.vector.reciprocal_approx_fast(
                    rec.rearrange("p m n -> p (m n)"),
                    epr[:, 0:2, :].rearrange("p m n -> p (m n)"),
                )
                tacc = outp.tile([P, 2, NODES_PER_SUP], BF16)
                nc.gpsimd.tensor_tensor(
                    out=tacc, in0=epr[:, 2:4, :], in1=rec,
                    op=mybir.AluOpType.mult,
                )

                # out[nodes, :] = t.T @ Wm' + bm'
                o_ps = psout.tile([P, C_OUT], F32, tag="o")
                for k in range(2):
                    nc.tensor.matmul(
                        o_ps, lhsT=tacc[:, k, :], rhs=wms[:, k, :],
                        start=(k == 0), stop=False,
                    )
                nc.tensor.matmul(
                    o_ps, lhsT=ones1, rhs=bmb, start=False, stop=True,
                )
                oo = outp.tile([P, C_OUT], F32)
                nc.scalar.copy(out=oo, in_=o_ps)
                nc.sync.dma_start(
                    out=out_d[s * NODES_PER_SUP:(s + 1) * NODES_PER_SUP, :], in_=oo
                )

    nc.compile()
    return nc


def _get_prog(nsup: int):
    if nsup not in _PROG_CACHE:
        _PROG_CACHE[nsup] = build_program(nsup)
    return _PROG_CACHE[nsup]


def prep_inputs(features, neighbor_idx, W1, b1, gamma, beta, run_mean, run_var,
                Ws, Wm, bm, n_cores=N_CORES):
    bf16 = ml_dtypes.bfloat16
    a = (gamma / np.sqrt(run_var + BN_EPS)).astype(np.float32)
    c = (beta - run_mean * a).astype(np.float32)
    wsp = (a[:, None] * Ws).astype(bf16)
    csv = (c @ Ws).astype(np.float32)
    wmp = (a[:, None] * Wm).astype(bf16)
    bmv = (c @ Wm + bm).astype(np.float32)

    tab = np.ascontiguousarray(features.astype(bf16).reshape(N_NODES // 2, 2 * C_IN))
    w1b = np.ascontiguousarray(W1.astype(bf16))
    b1c = np.ascontiguousarray(b1.astype(np.float32).reshape(2, P).T)
    csc = np.ascontiguousarray(csv.reshape(2, P).T)
    bmr = bmv.astype(bf16).reshape(1, C_OUT)

    n_pc = neighbor_idx.shape[0] // n_cores
    nodes_pc = -(-n_pc // NODES_PER_SUP) * NODES_PER_SUP
    nsup = nodes_pc // NODES_PER_SUP

    shared = dict(tab=tab, w1=w1b, wsp=np.ascontiguousarray(wsp),
                  wmp=np.ascontiguousarray(wmp), b1c=b1c, csc=csc, bmr=bmr)
    in_maps = []
    for ci in range(n_cores):
        ni = neighbor_idx[ci * n_pc:(ci + 1) * n_pc].astype(np.int64)
        if nodes_pc != n_pc:
            ni = np.concatenate(
                [ni, np.zeros((nodes_pc - n_pc, K_NBR), dtype=np.int64)], axis=0)
        flat = ni.reshape(-1)
        pair = (flat >> 1).astype(np.int16)
        parity = (flat & 1).astype(np.uint8)
        wrapped = pair.reshape(nsup, SUP // 16, 16)
        wrapped = np.transpose(wrapped, (0, 2, 1))
        idxs = np.tile(wrapped, (1, 8, 1)).reshape(nsup * P, SUP // 16)
        in_maps.append(dict(shared,
                            idxs=np.ascontiguousarray(idxs),
                            pmask=np.ascontiguousarray(parity.reshape(nsup, SUP))))
    return in_maps, nsup, n_pc


def kernel(**inputs):
    in_maps, nsup, n_pc = prep_inputs(**inputs)
    nc = _get_prog(nsup)
    res = run_bass_kernel_spmd(nc, in_maps, core_ids=list(range(N_CORES)))
    return np.concatenate([r["out"][:n_pc] for r in res.results], axis=0)


# revision 4
# speedup vs baseline: 1.1549x; 1.0527x over previous
"""LocalFeatureAggregation Trainium2 kernel (8 NeuronCores, data-parallel over nodes).

Algorithm (reference):
    x = relu(features @ W1 + b1)            # (N, 2C)
    g = BN(x[neighbor_idx])                 # (N, k, 2C), inference BN
    s = softmax(g @ Ws, axis=k)             # (N, k, 2C)
    out = (sum_k s * g) @ Wm + bm           # (N, C_out)

Device strategy (~2.15x faster than the tensor_reduce baseline):
  - BN folds into weights host-side; softmax needs no max-subtraction.
  - Data-parallel: core i handles 6250 nodes (padded to 6272).  Feature
    table replicated as bf16 DRAM pairs (int16 gather idx = node>>1), the
    odd/even 128-ch half selected on-chip with copy_predicated (DVE).
  - Channel-major per-sup pipeline (2048 gathered cols), engine-assigned
    by probing the CoreSim cost model (DVE ~94%, Pool ~92% busy):
      gather dispatch (Pool, hoisted 2 sups ahead) -> parity select (DVE)
      -> W1 matmul (PE) -> bias+relu evac (m0: Act, m1: DVE tensor_scalar)
      -> Ws matmul (PE) -> exp+bias evac (Act) -> pt = E*x (Pool)
      -> k=16 segmented sums as in-place bf16 tree-adds: L1 per half (DVE),
         L2 split h0->Pool / h1->DVE, L3 (DVE), final f32 level (Pool)
      -> reciprocal (DVE), t = psr*rec (Pool)
      -> out matmul + bias via 1-partition ones-row pass (PE)
      -> PSUM evac (Act copy) -> DMA out.
  - HW constraints honored: GPSIMD cannot access PSUM; matmul out f32;
    no divide opcode; gather calls <= 512 descriptors.
"""

import numpy as np
import ml_dtypes

import concourse.bass as bass
import concourse.bacc as bacc
import concourse.tile as tile
from concourse import mybir
from concourse.bass_utils import run_bass_kernel_spmd

BN_EPS = 1e-5
P = 128
N_NODES = 50000
K_NBR = 16
C_IN = 128
C2 = 256
C_OUT = 128
N_CORES = 8
NODES_PC_RAW = N_NODES // N_CORES
SUP = 2048
GCALL = 512
NODES_PER_SUP = SUP // K_NBR                # 128
HALF = 1024
BF16 = mybir.dt.bfloat16
F32 = mybir.dt.float32

_PROG_CACHE = {}


def build_program(nsup: int):
    nodes_pc = nsup * NODES_PER_SUP
    nc = bacc.Bacc("TRN2", num_devices=N_CORES)

    tab_d = nc.dram_tensor("tab", [N_NODES // 2, 2 * C_IN], BF16, kind="ExternalInput")
    idx_d = nc.dram_tensor("idxs", [nsup * P, SUP // 16], mybir.dt.int16,
                           kind="ExternalInput")
    pm_d = nc.dram_tensor("pmask", [nsup, SUP], mybir.dt.uint8, kind="ExternalInput")
    w1_d = nc.dram_tensor("w1", [C_IN, C2], BF16, kind="ExternalInput")
    ws_d = nc.dram_tensor("wsp", [C2, C2], BF16, kind="ExternalInput")
    wm_d = nc.dram_tensor("wmp", [C2, C_OUT], BF16, kind="ExternalInput")
    b1_d = nc.dram_tensor("b1c", [P, 2], F32, kind="ExternalInput")
    cs_d = nc.dram_tensor("csc", [P, 2], F32, kind="ExternalInput")
    bm_d = nc.dram_tensor("bmr", [1, C_OUT], BF16, kind="ExternalInput")
    out_d = nc.dram_tensor("out", [nodes_pc, C_OUT], F32, kind="ExternalOutput")

    with tile.TileContext(nc) as tc:
        with (
            tc.tile_pool(name="consts", bufs=1) as consts,
            tc.tile_pool(name="gsup", bufs=6) as gsup,
            tc.tile_pool(name="sel", bufs=16) as selp,
            tc.tile_pool(name="work", bufs=3) as work,
            tc.tile_pool(name="small", bufs=4) as small,
            tc.tile_pool(name="outp", bufs=3) as outp,
            tc.tile_pool(name="psum", bufs=3, space="PSUM") as psum,
            tc.tile_pool(name="psout", bufs=2, space="PSUM") as psout,
        ):
            # ---- constants -------------------------------------------------
            w1s = consts.tile([P, C2], BF16)
            nc.sync.dma_start(out=w1s, in_=w1_d[:, :])
            wss = consts.tile([P, 2, C2], BF16)
            for k in range(2):
                nc.sync.dma_start(out=wss[:, k, :], in_=ws_d[k * P:(k + 1) * P, :])
            wms = consts.tile([P, 2, C_OUT], BF16)
            for k in range(2):
                nc.sync.dma_start(out=wms[:, k, :], in_=wm_d[k * P:(k + 1) * P, :])
            b1c = consts.tile([P, 2], F32)
            nc.sync.dma_start(out=b1c, in_=b1_d[:, :])
            csc = consts.tile([P, 2], F32)
            nc.sync.dma_start(out=csc, in_=cs_d[:, :])
            bmb = consts.tile([1, C_OUT], BF16)
            nc.sync.dma_start(out=bmb, in_=bm_d[:, :])
            ones1 = consts.tile([1, P], BF16)
            nc.vector.memset(ones1, 1.0)

            def do_gathers(s):
                idx_t = gsup.tile([P, SUP // 16], mybir.dt.int16)
                nc.sync.dma_start(out=idx_t, in_=idx_d[s * P:(s + 1) * P, :])
                pm_t = gsup.tile([P, SUP], mybir.dt.uint8)
                nc.sync.dma_start(
                    out=pm_t,
                    in_=bass.AP(tensor=pm_d, offset=s * SUP, ap=[[0, P], [1, SUP]]),
                )
                fgq = []
                for q in range(SUP // GCALL):
                    fg = selp.tile([P, 2, GCALL], BF16, tag="fg")
                    nc.gpsimd.dma_gather(
                        fg[:, :, :], tab_d[:, :],
                        idx_t[:, q * (GCALL // 16):(q + 1) * (GCALL // 16)],
                        num_idxs=GCALL, num_idxs_reg=GCALL, elem_size=2 * C_IN,
                        transpose=True,
                    )
                    nc.vector.copy_predicated(
                        fg[:, 0, :], pm_t[:, q * GCALL:(q + 1) * GCALL],
                        fg[:, 1, :],
                    )
                    fgq.append(fg)
                return fgq

            fg_pipe = [do_gathers(0), do_gathers(1)]
            for s in range(nsup):
                fgq = fg_pipe.pop(0)
                if s + 2 < nsup:
                    fg_pipe.append(do_gathers(s + 2))

                # combined tile: [p, (et_m0|et_m1|pt_m0|pt_m1), SUP]
                ept = work.tile([P, 4, SUP], BF16, tag="ept")
                xgt = work.tile([P, 2, SUP], BF16, tag="xgt")
                eptv = ept.rearrange("p w (n k) -> p w n k", k=K_NBR)

                for h in range(SUP // HALF):
                    cols = slice(h * HALF, (h + 1) * HALF)
                    ncol = slice(h * (HALF // K_NBR), (h + 1) * (HALF // K_NBR))
                    # x = relu(W1.T @ f + b1), channel-major, bf16
                    for m in range(2):
                        xg_ps = psum.tile([P, HALF], F32, tag="mm")
                        for q in range(0, HALF, 512):
                            c0 = h * HALF + q
                            nc.tensor.matmul(
                                xg_ps[:, q:q + 512],
                                lhsT=w1s[:, m * P:(m + 1) * P],
                                rhs=fgq[c0 // GCALL][:, 0, c0 % GCALL:c0 % GCALL + 512],
                                start=True, stop=True,
                            )
                        if m == 1:
                            nc.vector.tensor_scalar(
                                out=xgt[:, m, cols], in0=xg_ps,
                                scalar1=b1c[:, m:m + 1], scalar2=0.0,
                                op0=mybir.AluOpType.add, op1=mybir.AluOpType.max,
                            )
                        else:
                            nc.scalar.activation(
                                out=xgt[:, m, cols], in_=xg_ps,
                                func=mybir.ActivationFunctionType.Relu,
                                bias=b1c[:, m:m + 1], scale=1.0,
                            )
                    # logits -> E = exp(x @ Ws' + cs), bf16
                    for m in range(2):
                        l_ps = psum.tile([P, HALF], F32, tag="mm")
                        for q in range(0, HALF, 512):
                            for k in range(2):
                                nc.tensor.matmul(
                                    l_ps[:, q:q + 512],
                                    lhsT=wss[:, k, m * P:(m + 1) * P],
                                    rhs=xgt[:, k, h * HALF + q:h * HALF + q + 512],
                                    start=(k == 0), stop=(k == 1),
                                )
                        nc.scalar.activation(
                            out=ept[:, m, cols], in_=l_ps,
                            func=mybir.ActivationFunctionType.Exp,
                            bias=csc[:, m:m + 1], scale=1.0,
                        )
                    # pt = E * x  (Pool)
                    nc.gpsimd.tensor_tensor(
                        out=ept[:, 2:4, cols], in0=ept[:, 0:2, cols],
                        in1=xgt[:, :, cols], op=mybir.AluOpType.mult,
                    )
                    # tree L1 for this half (DVE): k 16 -> 8 in place
                    nc.vector.tensor_tensor(
                        out=eptv[:, :, ncol, 0:8], in0=eptv[:, :, ncol, 0:8],
                        in1=eptv[:, :, ncol, 8:16], op=mybir.AluOpType.add,
                    )

                # ---- remaining tree levels, f32 last -----------------------
                nm = NODES_PER_SUP // 2
                nc.gpsimd.tensor_tensor(
                    out=eptv[:, :, 0:nm, 0:4], in0=eptv[:, :, 0:nm, 0:4],
                    in1=eptv[:, :, 0:nm, 4:8], op=mybir.AluOpType.add,
                )
                nc.vector.tensor_tensor(
                    out=eptv[:, :, nm:, 0:4], in0=eptv[:, :, nm:, 0:4],
                    in1=eptv[:, :, nm:, 4:8], op=mybir.AluOpType.add,
                )
                nc.vector.tensor_tensor(
                    out=eptv[:, :, :, 0:2], in0=eptv[:, :, :, 0:2],
                    in1=eptv[:, :, :, 2:4], op=mybir.AluOpType.add,
                )
                epr = small.tile([P, 4, NODES_PER_SUP], F32, tag="epr")
                nc.gpsimd.tensor_tensor(
                    out=epr, in0=eptv[:, :, :, 0], in1=eptv[:, :, :, 1],
                    op=mybir.AluOpType.add,
                )
                rec = small.tile([P, 2, NODES_PER_SUP], F32, tag="rec")
                nc.vector.reciprocal_approx_fast(
                    rec.rearrange("p m n -> p (m n)"),
                    epr[:, 0:2, :].rearrange("p m n -> p (m n)"),
                )
                tacc = outp.tile([P, 2, NODES_PER_SUP], BF16)
                nc.gpsimd.tensor_tensor(
                    out=tacc, in0=epr[:, 2:4, :], in1=rec,
                    op=mybir.AluOpType.mult,
                )

                # out[nodes, :] = t.T @ Wm' + bm'
                o_ps = psout.tile([P, C_OUT], F32, tag="o")
                for k in range(2):
                    nc.tensor.matmul(
                        o_ps, lhsT=tacc[:, k, :], rhs=wms[:, k, :],
                        start=(k == 0), stop=False,
                    )
                nc.tensor.matmul(
                    o_ps, lhsT=ones1, rhs=bmb, start=False, stop=True,
                )
                oo = outp.tile([P, C_OUT], F32)
                nc.scalar.copy(out=oo, in_=o_ps)
                nc.sync.dma_start(
                    out=out_d[s * NODES_PER_SUP:(s + 1) * NODES_PER_SUP, :], in_=oo
                )

    nc.compile()
    return nc


def _get_prog(nsup: int):
    if nsup not in _PROG_CACHE:
        _PROG_CACHE[nsup] = build_program(nsup)
    return _PROG_CACHE[nsup]


def prep_inputs(features, neighbor_idx, W1, b1, gamma, beta, run_mean, run_var,
                Ws, Wm, bm, n_cores=N_CORES):
    bf16 = ml_dtypes.bfloat16
    a = (gamma / np.sqrt(run_var + BN_EPS)).astype(np.float32)
    c = (beta - run_mean * a).astype(np.float32)
    wsp = (a[:, None] * Ws).astype(bf16)
    csv = (c @ Ws).astype(np.float32)
    wmp = (a[:, None] * Wm).astype(bf16)
    bmv = (c @ Wm + bm).astype(np.float32)

    tab = np.ascontiguousarray(features.astype(bf16).reshape(N_NODES // 2, 2 * C_IN))
    w1b = np.ascontiguousarray(W1.astype(bf16))
    b1c = np.ascontiguousarray(b1.astype(np.float32).reshape(2, P).T)
    csc = np.ascontiguousarray(csv.reshape(2, P).T)
    bmr = bmv.astype(bf16).reshape(1, C_OUT)

    n_pc = neighbor_idx.shape[0] // n_cores
    nodes_pc = -(-n_pc // NODES_PER_SUP) * NODES_PER_SUP
    nsup = nodes_pc // NODES_PER_SUP

    shared = dict(tab=tab, w1=w1b, wsp=np.ascontiguousarray(wsp),
                  wmp=np.ascontiguousarray(wmp), b1c=b1c, csc=csc, bmr=bmr)
    in_maps = []
    for ci in range(n_cores):
        ni = neighbor_idx[ci * n_pc:(ci + 1) * n_pc].astype(np.int64)
        if nodes_pc != n_pc:
            ni = np.concatenate(
                [ni, np.zeros((nodes_pc - n_pc, K_NBR), dtype=np.int64)], axis=0)
        flat = ni.reshape(-1)
        pair = (flat >> 1).astype(np.int16)
        parity = (flat & 1).astype(np.uint8)
        wrapped = pair.reshape(nsup, SUP // 16, 16)
        wrapped = np.transpose(wrapped, (0, 2, 1))
        idxs = np.tile(wrapped, (1, 8, 1)).reshape(nsup * P, SUP // 16)
        in_maps.append(dict(shared,
                            idxs=np.ascontiguousarray(idxs),
                            pmask=np.ascontiguousarray(parity.reshape(nsup, SUP))))
    return in_maps, nsup, n_pc


def kernel(**inputs):
    in_maps, nsup, n_pc = prep_inputs(**inputs)
    nc = _get_prog(nsup)
    res = run_bass_kernel_spmd(nc, in_maps, core_ids=list(range(N_CORES)))
    return np.concatenate([r["out"][:n_pc] for r in res.results], axis=0)


# revision 5
# speedup vs baseline: 1.1580x; 1.0027x over previous
"""LocalFeatureAggregation Trainium2 kernel (8 NeuronCores, data-parallel over nodes).

Algorithm (reference):
    x = relu(features @ W1 + b1)            # (N, 2C)
    g = BN(x[neighbor_idx])                 # (N, k, 2C), inference BN
    s = softmax(g @ Ws, axis=1)             # (N, k, 2C)
    out = (sum_k s * g) @ Wm + bm           # (N, C_out)

Device strategy (~2.16x faster than the tensor_reduce baseline):
  - BN folds into weights host-side; softmax needs no max-subtraction.
  - Data-parallel: core i handles 6250 nodes (padded to 6272).  Feature
    table replicated as bf16 DRAM pairs (int16 gather idx = node>>1), the
    odd/even 128-ch half selected on-chip with copy_predicated (DVE).
  - Channel-major per-sup pipeline (2048 gathered cols), engine-assigned
    by probing the CoreSim cost model (Pool ~95%, DVE ~93% busy):
      gather dispatch (Pool, hoisted 2 sups ahead) -> parity select (DVE)
      -> W1 matmul (PE) -> bias+relu evac (m0: Act, m1: DVE tensor_scalar)
      -> Ws matmul (PE) -> exp+bias evac (Act) -> pt = E*x (Pool)
      -> k=16 segmented sums as in-place bf16 tree-adds: L1 per half (DVE),
         L2 split 80 nodes->Pool / 48->DVE, L3 (DVE), final f32 level (Pool)
      -> reciprocal (DVE), t = psr*rec (Pool)
      -> out matmul + bias via 1-partition ones-row pass (PE)
      -> PSUM evac (Act copy) -> DMA out.
  - HW constraints honored: GPSIMD cannot access PSUM; matmul out f32;
    no divide opcode; gather calls <= 512 descriptors.
"""

import numpy as np
import ml_dtypes

import concourse.bass as bass
import concourse.bacc as bacc
import concourse.tile as tile
from concourse import mybir
from concourse.bass_utils import run_bass_kernel_spmd

BN_EPS = 1e-5
P = 128
N_NODES = 50000
K_NBR = 16
C_IN = 128
C2 = 256
C_OUT = 128
N_CORES = 8
NODES_PC_RAW = N_NODES // N_CORES
SUP = 2048
GCALL = 512
NODES_PER_SUP = SUP // K_NBR                # 128
HALF = 1024
BF16 = mybir.dt.bfloat16
F32 = mybir.dt.float32

_PROG_CACHE = {}


def build_program(nsup: int):
    nodes_pc = nsup * NODES_PER_SUP
    nc = bacc.Bacc("TRN2", num_devices=N_CORES)

    tab_d = nc.dram_tensor("tab", [N_NODES // 2, 2 * C_IN], BF16, kind="ExternalInput")
    idx_d = nc.dram_tensor("idxs", [nsup * P, SUP // 16], mybir.dt.int16,
                           kind="ExternalInput")
    pm_d = nc.dram_tensor("pmask", [nsup, SUP], mybir.dt.uint8, kind="ExternalInput")
    w1_d = nc.dram_tensor("w1", [C_IN, C2], BF16, kind="ExternalInput")
    ws_d = nc.dram_tensor("wsp", [C2, C2], BF16, kind="ExternalInput")
    wm_d = nc.dram_tensor("wmp", [C2, C_OUT], BF16, kind="ExternalInput")
    b1_d = nc.dram_tensor("b1c", [P, 2], F32, kind="ExternalInput")
    cs_d = nc.dram_tensor("csc", [P, 2], F32, kind="ExternalInput")
    bm_d = nc.dram_tensor("bmr", [1, C_OUT], BF16, kind="ExternalInput")
    out_d = nc.dram_tensor("out", [nodes_pc, C_OUT], F32, kind="ExternalOutput")

    with tile.TileContext(nc) as tc:
        with (
            tc.tile_pool(name="consts", bufs=1) as consts,
            tc.tile_pool(name="gsup", bufs=6) as gsup,
            tc.tile_pool(name="sel", bufs=16) as selp,
            tc.tile_pool(name="work", bufs=3) as work,
            tc.tile_pool(name="small", bufs=4) as small,
            tc.tile_pool(name="outp", bufs=3) as outp,
            tc.tile_pool(name="psum", bufs=3, space="PSUM") as psum,
            tc.tile_pool(name="psout", bufs=2, space="PSUM") as psout,
        ):
            # ---- constants -------------------------------------------------
            w1s = consts.tile([P, C2], BF16)
            nc.sync.dma_start(out=w1s, in_=w1_d[:, :])
            wss = consts.tile([P, 2, C2], BF16)
            for k in range(2):
                nc.sync.dma_start(out=wss[:, k, :], in_=ws_d[k * P:(k + 1) * P, :])
            wms = consts.tile([P, 2, C_OUT], BF16)
            for k in range(2):
                nc.sync.dma_start(out=wms[:, k, :], in_=wm_d[k * P:(k + 1) * P, :])
            b1c = consts.tile([P, 2], F32)
            nc.sync.dma_start(out=b1c, in_=b1_d[:, :])
            csc = consts.tile([P, 2], F32)
            nc.sync.dma_start(out=csc, in_=cs_d[:, :])
            bmb = consts.tile([1, C_OUT], BF16)
            nc.sync.dma_start(out=bmb, in_=bm_d[:, :])
            ones1 = consts.tile([1, P], BF16)
            nc.vector.memset(ones1, 1.0)

            def do_gathers(s):
                idx_t = gsup.tile([P, SUP // 16], mybir.dt.int16)
                nc.sync.dma_start(out=idx_t, in_=idx_d[s * P:(s + 1) * P, :])
                pm_t = gsup.tile([P, SUP], mybir.dt.uint8)
                nc.sync.dma_start(
                    out=pm_t,
                    in_=bass.AP(tensor=pm_d, offset=s * SUP, ap=[[0, P], [1, SUP]]),
                )
                fgq = []
                for q in range(SUP // GCALL):
                    fg = selp.tile([P, 2, GCALL], BF16, tag="fg")
                    nc.gpsimd.dma_gather(
                        fg[:, :, :], tab_d[:, :],
                        idx_t[:, q * (GCALL // 16):(q + 1) * (GCALL // 16)],
                        num_idxs=GCALL, num_idxs_reg=GCALL, elem_size=2 * C_IN,
                        transpose=True,
                    )
                    nc.vector.copy_predicated(
                        fg[:, 0, :], pm_t[:, q * GCALL:(q + 1) * GCALL],
                        fg[:, 1, :],
                    )
                    fgq.append(fg)
                return fgq

            fg_pipe = [do_gathers(0), do_gathers(1)]
            for s in range(nsup):
                fgq = fg_pipe.pop(0)
                if s + 2 < nsup:
                    fg_pipe.append(do_gathers(s + 2))

                # combined tile: [p, (et_m0|et_m1|pt_m0|pt_m1), SUP]
                ept = work.tile([P, 4, SUP], BF16, tag="ept")
                xgt = work.tile([P, 2, SUP], BF16, tag="xgt")
                eptv = ept.rearrange("p w (n k) -> p w n k", k=K_NBR)

                for h in range(SUP // HALF):
                    cols = slice(h * HALF, (h + 1) * HALF)
                    ncol = slice(h * (HALF // K_NBR), (h + 1) * (HALF // K_NBR))
                    # x = relu(W1.T @ f + b1), channel-major, bf16
                    for m in range(2):
                        xg_ps = psum.tile([P, HALF], F32, tag="mm")
                        for q in range(0, HALF, 512):
                            c0 = h * HALF + q
                            nc.tensor.matmul(
                                xg_ps[:, q:q + 512],
                                lhsT=w1s[:, m * P:(m + 1) * P],
                                rhs=fgq[c0 // GCALL][:, 0, c0 % GCALL:c0 % GCALL + 512],
                                start=True, stop=True,
                            )
                        if m == 1:
                            nc.vector.tensor_scalar(
                                out=xgt[:, m, cols], in0=xg_ps,
                                scalar1=b1c[:, m:m + 1], scalar2=0.0,
                                op0=mybir.AluOpType.add, op1=mybir.AluOpType.max,
                            )
                        else:
                            nc.scalar.activation(
                                out=xgt[:, m, cols], in_=xg_ps,
                                func=mybir.ActivationFunctionType.Relu,
                                bias=b1c[:, m:m + 1], scale=1.0,
                            )
                    # logits -> E = exp(x @ Ws' + cs), bf16
                    for m in range(2):
                        l_ps = psum.tile([P, HALF], F32, tag="mm")
                        for q in range(0, HALF, 512):
                            for k in range(2):
                                nc.tensor.matmul(
                                    l_ps[:, q:q + 512],
                                    lhsT=wss[:, k, m * P:(m + 1) * P],
                                    rhs=xgt[:, k, h * HALF + q:h * HALF + q + 512],
                                    start=(k == 0), stop=(k == 1),
                                )
                        nc.scalar.activation(
                            out=ept[:, m, cols], in_=l_ps,
                            func=mybir.ActivationFunctionType.Exp,
                            bias=csc[:, m:m + 1], scale=1.0,
                        )
                    # pt = E * x  (Pool)
                    nc.gpsimd.tensor_tensor(
                        out=ept[:, 2:4, cols], in0=ept[:, 0:2, cols],
                        in1=xgt[:, :, cols], op=mybir.AluOpType.mult,
                    )
                    # tree L1 for this half (DVE): k 16 -> 8 in place
                    nc.vector.tensor_tensor(
                        out=eptv[:, :, ncol, 0:8], in0=eptv[:, :, ncol, 0:8],
                        in1=eptv[:, :, ncol, 8:16], op=mybir.AluOpType.add,
                    )

                # ---- remaining tree levels, f32 last -----------------------
                nm = 80
                nc.gpsimd.tensor_tensor(
                    out=eptv[:, :, 0:nm, 0:4], in0=eptv[:, :, 0:nm, 0:4],
                    in1=eptv[:, :, 0:nm, 4:8], op=mybir.AluOpType.add,
                )
                nc.vector.tensor_tensor(
                    out=eptv[:, :, nm:, 0:4], in0=eptv[:, :, nm:, 0:4],
                    in1=eptv[:, :, nm:, 4:8], op=mybir.AluOpType.add,
                )
                nc.vector.tensor_tensor(
                    out=eptv[:, :, :, 0:2], in0=eptv[:, :, :, 0:2],
                    in1=eptv[:, :, :, 2:4], op=mybir.AluOpType.add,
                )
                epr = small.tile([P, 4, NODES_PER_SUP], F32, tag="epr")
                nc.gpsimd.tensor_tensor(
                    out=epr, in0=eptv[:, :, :, 0], in1=eptv[:, :, :, 1],
                    op=mybir.AluOpType.add,
                )
                rec = small.tile([P, 2, NODES_PER_SUP], F32, tag="rec")
                nc.vector.reciprocal_approx_fast(
                    rec.rearrange("p m n -> p (m n)"),
                    epr[:, 0:2, :].rearrange("p m n -> p (m n)"),
                )
                tacc = outp.tile([P, 2, NODES_PER_SUP], BF16)
                nc.gpsimd.tensor_tensor(
                    out=tacc, in0=epr[:, 2:4, :], in1=rec,
                    op=mybir.AluOpType.mult,
                )

                # out[nodes, :] = t.T @ Wm' + bm'
                o_ps = psout.tile([P, C_OUT], F32, tag="o")
                for k in range(2):
                    nc.tensor.matmul(
                        o_ps, lhsT=tacc[:, k, :], rhs=wms[:, k, :],
                        start=(k == 0), stop=False,
                    )
                nc.tensor.matmul(
                    o_ps, lhsT=ones1, rhs=bmb, start=False, stop=True,
                )
                oo = outp.tile([P, C_OUT], F32)
                nc.scalar.copy(out=oo, in_=o_ps)
                nc.sync.dma_start(
                    out=out_d[s * NODES_PER_SUP:(s + 1) * NODES_PER_SUP, :], in_=oo
                )

    nc.compile()
    return nc


def _get_prog(nsup: int):
    if nsup not in _PROG_CACHE:
        _PROG_CACHE[nsup] = build_program(nsup)
    return _PROG_CACHE[nsup]


def prep_inputs(features, neighbor_idx, W1, b1, gamma, beta, run_mean, run_var,
                Ws, Wm, bm, n_cores=N_CORES):
    bf16 = ml_dtypes.bfloat16
    a = (gamma / np.sqrt(run_var + BN_EPS)).astype(np.float32)
    c = (beta - run_mean * a).astype(np.float32)
    wsp = (a[:, None] * Ws).astype(bf16)
    csv = (c @ Ws).astype(np.float32)
    wmp = (a[:, None] * Wm).astype(bf16)
    bmv = (c @ Wm + bm).astype(np.float32)

    tab = np.ascontiguousarray(features.astype(bf16).reshape(N_NODES // 2, 2 * C_IN))
    w1b = np.ascontiguousarray(W1.astype(bf16))
    b1c = np.ascontiguousarray(b1.astype(np.float32).reshape(2, P).T)
    csc = np.ascontiguousarray(csv.reshape(2, P).T)
    bmr = bmv.astype(bf16).reshape(1, C_OUT)

    n_pc = neighbor_idx.shape[0] // n_cores
    nodes_pc = -(-n_pc // NODES_PER_SUP) * NODES_PER_SUP
    nsup = nodes_pc // NODES_PER_SUP

    shared = dict(tab=tab, w1=w1b, wsp=np.ascontiguousarray(wsp),
                  wmp=np.ascontiguousarray(wmp), b1c=b1c, csc=csc, bmr=bmr)
    in_maps = []
    for ci in range(n_cores):
        ni = neighbor_idx[ci * n_pc:(ci + 1) * n_pc].astype(np.int64)
        if nodes_pc != n_pc:
            ni = np.concatenate(
                [ni, np.zeros((nodes_pc - n_pc, K_NBR), dtype=np.int64)], axis=0)
        flat = ni.reshape(-1)
        pair = (flat >> 1).astype(np.int16)
        parity = (flat & 1).astype(np.uint8)
        wrapped = pair.reshape(nsup, SUP // 16, 16)
        wrapped = np.transpose(wrapped, (0, 2, 1))
        idxs = np.tile(wrapped, (1, 8, 1)).reshape(nsup * P, SUP // 16)
        in_maps.append(dict(shared,
                            idxs=np.ascontiguousarray(idxs),
                            pmask=np.ascontiguousarray(parity.reshape(nsup, SUP))))
    return in_maps, nsup, n_pc


def kernel(**inputs):
    in_maps, nsup, n_pc = prep_inputs(**inputs)
    nc = _get_prog(nsup)
    res = run_bass_kernel_spmd(nc, in_maps, core_ids=list(range(N_CORES)))
    return np.concatenate([r["out"][:n_pc] for r in res.results], axis=0)
